# revision 25
# baseline (speedup 1.0000x reference)
"""Trainium2 Bass kernel: ConvNeXt MLP + parallel MoE-LoRA (data-parallel over tokens).

Math per token t (D=512, Dh=2048, E=3 experts, r=8, top-k=2):
    base = gelu(x @ W1 + b1) @ W2 + b2
    g_e  = gelu(x @ w_down[e]) * wts[e, t]          (wts from top-k routing)
    out  = base + sum_e g_e @ w_up[e]

Strategy (per NeuronCore, 8 cores data-parallel on the token dim):
  - all operands are pre-packed on the host into their SBUF layouts in bf16:
    x arrives pre-transposed as xT [D, T_core] so the PE never runs x
    transposes and the DVE never runs casts; weights arrive in matmul-ready
    chunk layouts; b2 arrives pre-replicated [128, 512].
  - tokens tiled 128 at a time; supergroups of 4 tiles (512 tokens) so the
    MM1 moving free dim is 512.
  - MM1: hT[h,t] = W1_chunk.T @ xT (feature-major hidden, 4 PSUM banks so
    the Gelu drain never stalls the accumulation cadence), fused bias+gelu
    on ScalarE into actT (bf16).
  - MM2: out[t,d] accumulates 16 h-chunks (lhsT = actT slices) + the
    MoE-LoRA rank-24 matmul in one PSUM accumulation group; b2 added during
    the PSUM->SBUF drain from the preloaded replicated bias tile.
  - LoRA for supergroup g+1 is computed during MM2 of supergroup g (down
    proj -> gelu -> routing scale -> PE transpose), so the close matmul of
    MM2 never waits on the scalar/vector chain.
  - routing weights wts[e,t] = sum_k probs[t,k]*(idx[t,k]==e) computed on
    device in a small DVE prologue over all tokens at once.
  - W1 is DMA'd in h-quarter pieces so MM1 starts after ~1MB of weight
    traffic; the Gelu activation table is pre-warmed by a dummy activation.
"""

import os
import numpy as np

P = 128
D = 512
DH = 2048
E = 3
R = 8
ER = E * R  # 24
NH = DH // P  # 16
NDC = D // P  # 4 d-chunks
N_CORES = 8
T_FULL = 64 * 28 * 28  # 50176
TC = T_FULL // N_CORES  # 6272
NT = TC // P  # 49 token tiles
GT = 4  # tiles per supergroup

_CACHE = {}


def _build():
    import concourse.bacc as bacc
    import concourse.tile as tile
    import concourse.mybir as mybir
    from contextlib import ExitStack

    f32 = mybir.dt.float32
    bf16 = mybir.dt.bfloat16
    i32 = mybir.dt.int32
    AF = mybir.ActivationFunctionType
    OP = mybir.AluOpType

    nt = NT
    groups = []
    t0 = 0
    while t0 < nt:
        ng = min(GT, nt - t0)
        groups.append((t0, ng))
        t0 += ng
    ngrp = len(groups)

    nc = bacc.Bacc("TRN2", target_bir_lowering=False, debug=False,
                   num_devices=N_CORES)

    xt = nc.dram_tensor("xt", [P, NDC * TC], bf16, kind="ExternalInput").ap()
    w1 = nc.dram_tensor("w1", [P, 4 * DH], bf16, kind="ExternalInput").ap()
    w2 = nc.dram_tensor("w2", [P, NH * D], bf16, kind="ExternalInput").ap()
    wd = nc.dram_tensor("wd", [P, NDC * ER], bf16, kind="ExternalInput").ap()
    wu = nc.dram_tensor("wu", [P, D], bf16, kind="ExternalInput").ap()
    b1 = nc.dram_tensor("b1", [P, NH], f32, kind="ExternalInput").ap()
    b2 = nc.dram_tensor("b2", [P, D], f32, kind="ExternalInput").ap()
    tkp = nc.dram_tensor("tkp", [P, NT * 2], f32, kind="ExternalInput").ap()
    tki = nc.dram_tensor("tki", [P, NT * 4], i32, kind="ExternalInput").ap()
    ident_d = nc.dram_tensor("ident", [P, P], bf16, kind="ExternalInput").ap()
    out = nc.dram_tensor("out", [TC, D], f32, kind="ExternalOutput").ap()

    with tile.TileContext(nc) as tc, ExitStack() as ctx:
        cons = ctx.enter_context(tc.tile_pool(name="cons", bufs=1))
        xtp = ctx.enter_context(tc.tile_pool(name="xtp", bufs=4))
        actp = ctx.enter_context(tc.tile_pool(name="actp", bufs=2))
        outp = ctx.enter_context(tc.tile_pool(name="outp", bufs=4))
        lp = ctx.enter_context(tc.tile_pool(name="lp", bufs=4))
        gtp = ctx.enter_context(tc.tile_pool(name="gtp", bufs=8))
        ps_h = ctx.enter_context(tc.tile_pool(name="ps_h", bufs=3, space="PSUM"))
        ps_o = ctx.enter_context(tc.tile_pool(name="ps_o", bufs=2, space="PSUM"))
        ps_g = ctx.enter_context(tc.tile_pool(name="ps_g", bufs=1, space="PSUM"))
        ps_t = ctx.enter_context(tc.tile_pool(name="ps_t", bufs=1, space="PSUM"))

        # preload the Gelu activation table before the first real activation
        warm_in = cons.tile([1, 8], f32)
        nc.vector.memset(warm_in[:], 0.125)
        warm_out = cons.tile([1, 8], f32)
        nc.scalar.activation(warm_out[:], warm_in[:], AF.Gelu)
        # dummy matmul fodder: the PE clock ramps ~6us after sustained
        # activity starts, so burn the DMA wait on throwaway matmuls to
        # trigger the ramp before the real data lands
        dum_in = cons.tile([P, 512], bf16)
        nc.vector.memset(dum_in[:], 0.0)

        # ---- DMAs in issue order (earlier == higher effective priority).
        # All sources are host-packed so every DMA is one contiguous chunk
        # per partition (cheap descriptor generation on the sync engine).
        ident_sb = cons.tile([P, P], bf16)
        nc.sync.dma_start(ident_sb[:], ident_d)

        xT_tiles = {}

        def dma_xt(g):
            t0g, ngg = groups[g]
            G = ngg * P
            xTt = xtp.tile([P, NDC * G], bf16, tag="xT", name=f"xT{g}")
            nc.sync.dma_start(xTt[:], xt[:, NDC * t0g * P:NDC * (t0g + ngg) * P])
            xT_tiles[g] = xTt

        # first-needed data in small pieces so MM1 h-chunk 0 starts earliest:
        # W1 quarter 0 in four hh-pieces, xT group 0 in four c-pieces
        W1s = cons.tile([P, 4 * DH], bf16)
        xT0 = xtp.tile([P, NDC * GT * P], bf16, tag="xT", name="xT0")
        xT_tiles[0] = xT0
        b1s = cons.tile([P, NH], f32)
        wdr = cons.tile([P, NDC * ER], bf16)
        nc.sync.dma_start(W1s[:, 0:512], w1[:, 0:512])        # q0 hh0
        nc.sync.dma_start(xT0[:, 0:512], xt[:, 0:512])        # g0 c0
        nc.sync.dma_start(b1s[:], b1)
        nc.sync.dma_start(xT0[:, 512:1024], xt[:, 512:1024])  # g0 c1
        nc.sync.dma_start(W1s[:, 512:1024], w1[:, 512:1024])  # q0 hh1
        nc.sync.dma_start(xT0[:, 1024:1536], xt[:, 1024:1536])  # g0 c2
        nc.sync.dma_start(xT0[:, 1536:2048], xt[:, 1536:2048])  # g0 c3
        nc.sync.dma_start(W1s[:, 1024:2048], w1[:, 1024:2048])  # q0 hh2-3
        nc.sync.dma_start(wdr[:], wd)
        for q in (1, 2, 3):
            nc.sync.dma_start(W1s[:, q * DH:(q + 1) * DH],
                              w1[:, q * DH:(q + 1) * DH])
        tp_sb = cons.tile([P, nt * 2], f32)
        nc.sync.dma_start(tp_sb[:], tkp)
        ti_sb = cons.tile([P, nt * 4], i32)
        nc.sync.dma_start(ti_sb[:], tki)
        dma_xt(1)
        W2s = cons.tile([P, NH * D], bf16)
        for j in range(4):
            nc.sync.dma_start(W2s[:, j * 4 * D:(j + 1) * 4 * D],
                              w2[:, j * 4 * D:(j + 1) * 4 * D])
        # w_up replicated at partition offsets 0/32/64/96 so each tile's
        # close matmul has lhsT/rhs at the same (aligned) base partition
        wur = cons.tile([P, D], bf16)
        nc.sync.dma_start(wur[:], wu)
        b2rep = cons.tile([P, D], f32)
        nc.sync.dma_start(b2rep[:], b2)

        # ---- routing weights wts[p, e*nt + tile] (DVE) ----
        idxf = cons.tile([P, nt * 2], f32)
        iv = ti_sb.rearrange("p (n k two) -> p n k two", k=2, two=2)
        nc.vector.tensor_copy(
            idxf.rearrange("p (n k one) -> p n k one", k=2, one=1),
            iv[:, :, :, 0:1])
        wts = cons.tile([P, E * nt], f32)
        for e in range(E):
            eq = cons.tile([P, nt * 2], f32, tag="eq", name=f"eq{e}", bufs=2)
            nc.vector.tensor_scalar(eq[:], idxf[:], float(e), None,
                                    op0=OP.is_equal)
            nc.vector.tensor_tensor(eq[:], eq[:], tp_sb[:], op=OP.mult)
            nc.vector.tensor_reduce(wts[:, e * nt:(e + 1) * nt],
                                    eq.rearrange("p (n k) -> p n k", k=2),
                                    axis=mybir.AxisListType.X, op=OP.add)

        # ---- emit helpers ----
        gts = {}  # group -> list of [ER, P] bf16 lhsT tiles for the MM2 close

        def emit_mm1_chunk(g, h, actT):
            t0g, ngg = groups[g]
            G = ngg * P
            q, hh = h // 4, h % 4
            xTt = xT_tiles[g]
            ph = ps_h.tile([P, 512], f32, tag="ph", name=f"ph{g}_{h}")
            base = q * DH + hh * 512
            for c in range(NDC):
                nc.tensor.matmul(
                    ph[:, :G],
                    W1s[:, base + c * P:base + (c + 1) * P],
                    xTt[:, c * G:(c + 1) * G],
                    start=(c == 0), stop=(c == NDC - 1))
            nc.scalar.activation(actT[:, h * G:(h + 1) * G], ph[:, :G],
                                 AF.Gelu, bias=b1s[:, h:h + 1], scale=1.0)

        def emit_lora_down(g):
            t0g, ngg = groups[g]
            G = ngg * P
            xTt = xT_tiles[g]
            pga = ps_g.tile([P, GT * ER], f32, tag="pg", name=f"pg{g}")
            for i in range(ngg):
                for c in range(NDC):
                    nc.tensor.matmul(
                        pga[:, i * ER:(i + 1) * ER],
                        xTt[:, c * G + i * P:c * G + (i + 1) * P],
                        wdr[:, c * ER:(c + 1) * ER],
                        start=(c == 0), stop=(c == NDC - 1))
            g_sba = lp.tile([P, ngg * ER], f32, tag="g_sb", name=f"g_sb{g}")
            nc.scalar.activation(g_sba[:], pga[:, :ngg * ER], AF.Gelu)
            # g2 blocks padded to a 32-column stride so the transposed rows
            # land at base partitions 0/32/64/96 (PE tile-position rule)
            g2a = lp.tile([P, ngg * 32], bf16, tag="g2", name=f"g2{g}")
            for i in range(ngg):
                tt = t0g + i
                for e in range(E):
                    nc.vector.tensor_scalar(
                        g2a[:, i * 32 + e * R:i * 32 + (e + 1) * R],
                        g_sba[:, i * ER + e * R:i * ER + (e + 1) * R],
                        wts[:, e * nt + tt:e * nt + tt + 1], None,
                        op0=OP.mult)
            return g2a

        def emit_lora_t(g, g2a):
            # transpose two tiles at a time: [128, 64] -> [64, 128], so the
            # per-tile rows sit at base partition 0/32 (PE quadrant rule)
            t0g, ngg = groups[g]
            lst = []
            for h0 in range(0, ngg, 2):
                w = min(2, ngg - h0) * 32
                pta = ps_t.tile([w, P], bf16, tag="pt", name=f"pt{g}_{h0}",
                                bufs=2)
                nc.tensor.matmul(pta[:], g2a[:, h0 * 32:h0 * 32 + w],
                                 ident_sb[:], is_transpose=True)
                gta = gtp.tile([w, P], bf16, tag="gt", name=f"gt{g}_{h0}")
                nc.vector.tensor_copy(gta[:], pta[:])
                for j in range(w // 32):
                    lst.append(gta[j * 32:j * 32 + ER, :])
            gts[g] = lst

        def emit_mm2_tile(g, i, actT):
            t0g, ngg = groups[g]
            G = ngg * P
            tt = t0g + i
            po = ps_o.tile([P, D], f32, tag="po", name=f"po{tt}")
            # LoRA close first: its short lhsT load hides behind the
            # previous tile's stream instead of bubbling mid-accumulation
            nc.tensor.matmul(po[:], gts[g][i],
                             wur[(i % 2) * 32:(i % 2) * 32 + ER, :],
                             start=True, stop=False)
            for h in range(NH):
                nc.tensor.matmul(
                    po[:],
                    actT[:, h * G + i * P:h * G + (i + 1) * P],
                    W2s[:, h * D:(h + 1) * D],
                    start=False, stop=(h == NH - 1))
            o_sb = outp.tile([P, D], f32, tag="o_sb", name=f"o_sb{tt}")
            nc.vector.tensor_tensor(o_sb[:], po[:], b2rep[:], op=OP.add)
            nc.sync.dma_start(out[tt * P:(tt + 1) * P, :], o_sb[:])

        # ---- main loop, two-stage software pipeline on the PE:
        # MM1 of group g+1 is interleaved between MM2 tiles of group g, and
        # the LoRA chain for group g+1 also runs under MM2 of group g, so
        # neither a group boundary nor the final group ever stalls the PE.
        actTs = {}

        def make_actT(g):
            t0g, ngg = groups[g]
            a = actp.tile([P, NH * ngg * P], bf16, tag="actT", name=f"actT{g}")
            actTs[g] = a
            return a

        # PE warm-up: throwaway matmuls while the first DMAs land, so the
        # clock ramp (triggered by activity) completes before real work
        for k in range(5):
            pd = ps_o.tile([P, D], f32, tag="po", name=f"dum{k}")
            nc.tensor.matmul(pd[:], dum_in[:, 0:P], dum_in[:],
                             start=True, stop=True)

        # pipeline fill: MM1(0), with group 0's LoRA folded in at points
        # where its inputs (wd, routing weights) have arrived
        a0 = make_actT(0)
        for h in range(NH):
            emit_mm1_chunk(0, h, a0)
            if h == 6:
                g0_g2s = emit_lora_down(0)
            if h == 12:
                emit_lora_t(0, g0_g2s)

        for g in range(ngrp):
            t0g, ngg = groups[g]
            actT = actTs[g]
            if g + 1 < ngrp:
                nxt = make_actT(g + 1)
                nxt_g2s = emit_lora_down(g + 1)
                emit_mm2_tile(g, 0, actT)
                emit_lora_t(g + 1, nxt_g2s)
                blocks = ((0, 5), (5, 10), (10, 16))
                for bi, i in enumerate(range(1, ngg)):
                    for h in range(*blocks[bi]):
                        emit_mm1_chunk(g + 1, h, nxt)
                    emit_mm2_tile(g, i, actT)
            else:
                for i in range(ngg):
                    emit_mm2_tile(g, i, actT)
            if g + 2 < ngrp:
                dma_xt(g + 2)

    nc.compile()
    return nc


def _get_nc():
    if "nc" not in _CACHE:
        _CACHE["nc"] = _build()
    return _CACHE["nc"]


def _make_in_maps(inputs):
    import ml_dtypes
    bf16 = ml_dtypes.bfloat16

    x = np.asarray(inputs["x"], dtype=np.float32)
    T = x.size // D
    x_flat = x.reshape(T, D)
    W1 = np.asarray(inputs["W1"], dtype=np.float32)
    W2 = np.asarray(inputs["W2"], dtype=np.float32)
    b1 = np.asarray(inputs["b1"], dtype=np.float32)
    b2 = np.asarray(inputs["b2"], dtype=np.float32)
    wdn = np.asarray(inputs["w_down"], dtype=np.float32)
    wup = np.asarray(inputs["w_up"], dtype=np.float32)
    tkp = np.ascontiguousarray(inputs["topk_probs"], dtype=np.float32)
    tki_in = np.asarray(inputs["topk_indices"])

    # SBUF layouts, bf16 (shared by all cores)
    # W1 [D, DH] -> [p, q, hh, c, x]: hh-major so the first DMA piece covers
    # the first MM1 h-chunks
    w1p = np.ascontiguousarray(
        W1.reshape(NDC, P, 4, 4, P).transpose(1, 2, 3, 0, 4).reshape(P, 4 * DH)
    ).astype(bf16)
    # W2 [DH, D] -> [p, n, d]
    w2p = np.ascontiguousarray(
        W2.reshape(NH, P, D).transpose(1, 0, 2).reshape(P, NH * D)).astype(bf16)
    # w_down [E, D, R] -> [D, E*R] -> [p, c, er]
    wdf = wdn.transpose(1, 0, 2).reshape(D, ER)
    wdp = np.ascontiguousarray(
        wdf.reshape(NDC, P, ER).transpose(1, 0, 2).reshape(P, NDC * ER)
    ).astype(bf16)
    wup_b = np.zeros((P, D), dtype=bf16)
    for i in range(GT):
        wup_b[i * 32:i * 32 + ER, :] = wup.reshape(ER, D).astype(bf16)
    b1p = np.ascontiguousarray(b1.reshape(NH, P).T)
    b2p = np.ascontiguousarray(np.broadcast_to(b2[None, :], (P, D)),
                               dtype=np.float32)
    ident = np.eye(P, dtype=np.float32).astype(bf16)

    groups = []
    t0 = 0
    while t0 < NT:
        ng = min(GT, NT - t0)
        groups.append((t0, ng))
        t0 += ng

    in_maps = []
    for c in range(N_CORES):
        sl = slice(c * TC, (c + 1) * TC)
        # x.T in bf16, regrouped per supergroup to [p, (group: c, t)] so the
        # per-group DMA is one contiguous chunk per partition
        xc = x_flat[sl].T.astype(bf16)          # [D, TC] = [(c p), t]
        xr = xc.reshape(NDC, P, TC)
        parts = [
            np.ascontiguousarray(
                xr[:, :, t0g * P:(t0g + ngg) * P].transpose(1, 0, 2)
            ).reshape(P, -1)
            for t0g, ngg in groups
        ]
        xt_c = np.ascontiguousarray(np.concatenate(parts, axis=1))
        # routing tensors packed to [p, n, k] (token tile n, partition p)
        tkp_c = np.ascontiguousarray(
            tkp[sl].reshape(NT, P, 2).transpose(1, 0, 2)).reshape(P, NT * 2)
        tki4 = np.zeros((TC, 4), dtype=np.int32)
        tki4[:, 0] = tki_in[sl, 0]
        tki4[:, 2] = tki_in[sl, 1]
        tki_c = np.ascontiguousarray(
            tki4.reshape(NT, P, 4).transpose(1, 0, 2)).reshape(P, NT * 4)
        in_maps.append(dict(
            xt=xt_c, w1=w1p, w2=w2p, wd=wdp, wu=wup_b, b1=b1p, b2=b2p,
            tkp=tkp_c, tki=tki_c, ident=ident))
    return in_maps


def _ensure_ntff_hook():
    """Register the axon NTFF profile hook if the image's antenv lacks it."""
    import sys
    import types
    try:
        from antenv.axon_hooks import get_axon_ntff_profile_hook  # noqa: F401
        return True
    except ImportError:
        pass
    try:
        from trn_agent_boot.trn_boot import _ntff_profile_via_ctypes
        mod = types.ModuleType("antenv.axon_hooks")
        _hook = [None]
        mod.set_axon_ntff_profile_hook = lambda h: _hook.__setitem__(0, h)
        mod.get_axon_ntff_profile_hook = lambda: _hook[0]
        sys.modules["antenv.axon_hooks"] = mod
        import antenv
        antenv.axon_hooks = mod
        mod.set_axon_ntff_profile_hook(
            _ntff_profile_via_ctypes("/opt/axon/libaxon_pjrt.so"))
        return True
    except Exception:
        return False


def kernel(**inputs):
    from concourse.bass_utils import run_bass_kernel_spmd

    nc = _get_nc()
    in_maps = _make_in_maps(inputs)
    trace = bool(int(os.environ.get("KERNEL_TRACE", "0")))
    if trace and not _ensure_ntff_hook():
        trace = False
    res = run_bass_kernel_spmd(nc, in_maps, list(range(N_CORES)), trace=trace)
    if trace:
        _CACHE["last_result"] = res
    out = np.concatenate([res.results[i]["out"] for i in range(N_CORES)], axis=0)
    return out.reshape(np.asarray(inputs["x"]).shape).astype(np.float32)


# revision 26
# speedup vs baseline: 1.0035x; 1.0035x over previous
"""Trainium2 Bass kernel: ConvNeXt MLP + parallel MoE-LoRA (data-parallel over tokens).

Math per token t (D=512, Dh=2048, E=3 experts, r=8, top-k=2):
    base = gelu(x @ W1 + b1) @ W2 + b2
    g_e  = gelu(x @ w_down[e]) * wts[e, t]          (wts from top-k routing)
    out  = base + sum_e g_e @ w_up[e]

Strategy (per NeuronCore, 8 cores data-parallel on the token dim):
  - all operands are pre-packed on the host into their SBUF layouts in bf16:
    x arrives pre-transposed as xT [D, T_core] so the PE never runs x
    transposes and the DVE never runs casts; weights arrive in matmul-ready
    chunk layouts; b2 arrives pre-replicated [128, 512].
  - tokens tiled 128 at a time; supergroups of 4 tiles (512 tokens) so the
    MM1 moving free dim is 512.
  - MM1: hT[h,t] = W1_chunk.T @ xT (feature-major hidden, 4 PSUM banks so
    the Gelu drain never stalls the accumulation cadence), fused bias+gelu
    on ScalarE into actT (bf16).
  - MM2: out[t,d] accumulates 16 h-chunks (lhsT = actT slices) + the
    MoE-LoRA rank-24 matmul in one PSUM accumulation group; b2 added during
    the PSUM->SBUF drain from the preloaded replicated bias tile.
  - LoRA for supergroup g+1 is computed during MM2 of supergroup g (down
    proj -> gelu -> routing scale -> PE transpose), so the close matmul of
    MM2 never waits on the scalar/vector chain.
  - routing weights wts[e,t] = sum_k probs[t,k]*(idx[t,k]==e) computed on
    device in a small DVE prologue over all tokens at once.
  - W1 is DMA'd in h-quarter pieces so MM1 starts after ~1MB of weight
    traffic; the Gelu activation table is pre-warmed by a dummy activation.
"""

import os
import numpy as np

P = 128
D = 512
DH = 2048
E = 3
R = 8
ER = E * R  # 24
NH = DH // P  # 16
NDC = D // P  # 4 d-chunks
N_CORES = 8
T_FULL = 64 * 28 * 28  # 50176
TC = T_FULL // N_CORES  # 6272
NT = TC // P  # 49 token tiles
GT = 4  # tiles per supergroup

_CACHE = {}


def _build():
    import concourse.bacc as bacc
    import concourse.tile as tile
    import concourse.mybir as mybir
    from contextlib import ExitStack

    f32 = mybir.dt.float32
    bf16 = mybir.dt.bfloat16
    i32 = mybir.dt.int32
    AF = mybir.ActivationFunctionType
    OP = mybir.AluOpType

    nt = NT
    groups = []
    t0 = 0
    while t0 < nt:
        ng = min(GT, nt - t0)
        groups.append((t0, ng))
        t0 += ng
    ngrp = len(groups)

    nc = bacc.Bacc("TRN2", target_bir_lowering=False, debug=False,
                   num_devices=N_CORES)

    xt = nc.dram_tensor("xt", [P, NDC * TC], bf16, kind="ExternalInput").ap()
    w1 = nc.dram_tensor("w1", [P, 4 * DH], bf16, kind="ExternalInput").ap()
    w2 = nc.dram_tensor("w2", [P, NH * D], bf16, kind="ExternalInput").ap()
    wd = nc.dram_tensor("wd", [P, NDC * ER], bf16, kind="ExternalInput").ap()
    wu = nc.dram_tensor("wu", [P, D], bf16, kind="ExternalInput").ap()
    b1 = nc.dram_tensor("b1", [P, NH], f32, kind="ExternalInput").ap()
    b2 = nc.dram_tensor("b2", [P, D], f32, kind="ExternalInput").ap()
    tkp = nc.dram_tensor("tkp", [P, NT * 2], f32, kind="ExternalInput").ap()
    tki = nc.dram_tensor("tki", [P, NT * 4], i32, kind="ExternalInput").ap()
    ident_d = nc.dram_tensor("ident", [P, P], bf16, kind="ExternalInput").ap()
    out = nc.dram_tensor("out", [TC, D], f32, kind="ExternalOutput").ap()

    with tile.TileContext(nc) as tc, ExitStack() as ctx:
        cons = ctx.enter_context(tc.tile_pool(name="cons", bufs=1))
        xtp = ctx.enter_context(tc.tile_pool(name="xtp", bufs=4))
        actp = ctx.enter_context(tc.tile_pool(name="actp", bufs=2))
        outp = ctx.enter_context(tc.tile_pool(name="outp", bufs=4))
        lp = ctx.enter_context(tc.tile_pool(name="lp", bufs=4))
        gtp = ctx.enter_context(tc.tile_pool(name="gtp", bufs=8))
        ps_h = ctx.enter_context(tc.tile_pool(name="ps_h", bufs=3, space="PSUM"))
        ps_o = ctx.enter_context(tc.tile_pool(name="ps_o", bufs=2, space="PSUM"))
        ps_g = ctx.enter_context(tc.tile_pool(name="ps_g", bufs=1, space="PSUM"))
        ps_t = ctx.enter_context(tc.tile_pool(name="ps_t", bufs=1, space="PSUM"))

        # preload the Gelu activation table before the first real activation
        warm_in = cons.tile([1, 8], f32)
        nc.vector.memset(warm_in[:], 0.125)
        warm_out = cons.tile([1, 8], f32)
        nc.scalar.activation(warm_out[:], warm_in[:], AF.Gelu)
        # dummy matmul fodder: the PE clock ramps ~6us after sustained
        # activity starts, so burn the DMA wait on throwaway matmuls to
        # trigger the ramp before the real data lands
        dum_in = cons.tile([P, 512], bf16)
        nc.vector.memset(dum_in[:], 0.0)

        # ---- DMAs in issue order (earlier == higher effective priority).
        # All sources are host-packed so every DMA is one contiguous chunk
        # per partition (cheap descriptor generation on the sync engine).
        ident_sb = cons.tile([P, P], bf16)
        nc.sync.dma_start(ident_sb[:], ident_d)

        xT_tiles = {}

        def dma_xt(g):
            t0g, ngg = groups[g]
            G = ngg * P
            xTt = xtp.tile([P, NDC * G], bf16, tag="xT", name=f"xT{g}")
            nc.sync.dma_start(xTt[:], xt[:, NDC * t0g * P:NDC * (t0g + ngg) * P])
            xT_tiles[g] = xTt

        # first-needed data in small pieces so MM1 h-chunk 0 starts earliest:
        # W1 quarter 0 in four hh-pieces, xT group 0 in four c-pieces
        W1s = cons.tile([P, 4 * DH], bf16)
        xT0 = xtp.tile([P, NDC * GT * P], bf16, tag="xT", name="xT0")
        xT_tiles[0] = xT0
        b1s = cons.tile([P, NH], f32)
        wdr = cons.tile([P, NDC * ER], bf16)
        nc.sync.dma_start(W1s[:, 0:512], w1[:, 0:512])        # q0 hh0
        nc.sync.dma_start(xT0[:, 0:512], xt[:, 0:512])        # g0 c0
        nc.sync.dma_start(b1s[:], b1)
        nc.sync.dma_start(xT0[:, 512:1024], xt[:, 512:1024])  # g0 c1
        nc.sync.dma_start(W1s[:, 512:1024], w1[:, 512:1024])  # q0 hh1
        nc.sync.dma_start(xT0[:, 1024:1536], xt[:, 1024:1536])  # g0 c2
        nc.sync.dma_start(xT0[:, 1536:2048], xt[:, 1536:2048])  # g0 c3
        nc.sync.dma_start(W1s[:, 1024:2048], w1[:, 1024:2048])  # q0 hh2-3
        nc.sync.dma_start(wdr[:], wd)
        for q in (1, 2, 3):
            nc.sync.dma_start(W1s[:, q * DH:(q + 1) * DH],
                              w1[:, q * DH:(q + 1) * DH])
        tp_sb = cons.tile([P, nt * 2], f32)
        nc.sync.dma_start(tp_sb[:], tkp)
        ti_sb = cons.tile([P, nt * 4], i32)
        nc.sync.dma_start(ti_sb[:], tki)
        dma_xt(1)
        W2s = cons.tile([P, NH * D], bf16)
        for j in range(4):
            nc.sync.dma_start(W2s[:, j * 4 * D:(j + 1) * 4 * D],
                              w2[:, j * 4 * D:(j + 1) * 4 * D])
        # w_up replicated at partition offsets 0/32/64/96 so each tile's
        # close matmul has lhsT/rhs at the same (aligned) base partition
        wur = cons.tile([P, D], bf16)
        nc.sync.dma_start(wur[:], wu)
        b2rep = cons.tile([P, D], f32)
        nc.sync.dma_start(b2rep[:], b2)

        # ---- routing weights wts[p, e*nt + tile] (DVE) ----
        idxf = cons.tile([P, nt * 2], f32)
        iv = ti_sb.rearrange("p (n k two) -> p n k two", k=2, two=2)
        nc.vector.tensor_copy(
            idxf.rearrange("p (n k one) -> p n k one", k=2, one=1),
            iv[:, :, :, 0:1])
        wts = cons.tile([P, E * nt], f32)
        for e in range(E):
            eq = cons.tile([P, nt * 2], f32, tag="eq", name=f"eq{e}", bufs=2)
            nc.vector.tensor_scalar(eq[:], idxf[:], float(e), None,
                                    op0=OP.is_equal)
            nc.vector.tensor_tensor(eq[:], eq[:], tp_sb[:], op=OP.mult)
            nc.vector.tensor_reduce(wts[:, e * nt:(e + 1) * nt],
                                    eq.rearrange("p (n k) -> p n k", k=2),
                                    axis=mybir.AxisListType.X, op=OP.add)

        # ---- emit helpers ----
        gts = {}  # group -> list of [ER, P] bf16 lhsT tiles for the MM2 close

        def emit_mm1_chunk(g, h, actT):
            t0g, ngg = groups[g]
            G = ngg * P
            q, hh = h // 4, h % 4
            xTt = xT_tiles[g]
            ph = ps_h.tile([P, 512], f32, tag="ph", name=f"ph{g}_{h}")
            base = q * DH + hh * 512
            for c in range(NDC):
                nc.tensor.matmul(
                    ph[:, :G],
                    W1s[:, base + c * P:base + (c + 1) * P],
                    xTt[:, c * G:(c + 1) * G],
                    start=(c == 0), stop=(c == NDC - 1))
            nc.scalar.activation(actT[:, h * G:(h + 1) * G], ph[:, :G],
                                 AF.Gelu, bias=b1s[:, h:h + 1], scale=1.0)

        def emit_lora_down(g):
            t0g, ngg = groups[g]
            G = ngg * P
            xTt = xT_tiles[g]
            pga = ps_g.tile([P, GT * ER], f32, tag="pg", name=f"pg{g}")
            for i in range(ngg):
                for c in range(NDC):
                    nc.tensor.matmul(
                        pga[:, i * ER:(i + 1) * ER],
                        xTt[:, c * G + i * P:c * G + (i + 1) * P],
                        wdr[:, c * ER:(c + 1) * ER],
                        start=(c == 0), stop=(c == NDC - 1))
            g_sba = lp.tile([P, ngg * ER], f32, tag="g_sb", name=f"g_sb{g}")
            nc.scalar.activation(g_sba[:], pga[:, :ngg * ER], AF.Gelu)
            # g2 blocks padded to a 32-column stride so the transposed rows
            # land at base partitions 0/32/64/96 (PE tile-position rule)
            g2a = lp.tile([P, ngg * 32], bf16, tag="g2", name=f"g2{g}")
            for i in range(ngg):
                tt = t0g + i
                for e in range(E):
                    nc.vector.tensor_scalar(
                        g2a[:, i * 32 + e * R:i * 32 + (e + 1) * R],
                        g_sba[:, i * ER + e * R:i * ER + (e + 1) * R],
                        wts[:, e * nt + tt:e * nt + tt + 1], None,
                        op0=OP.mult)
            return g2a

        def emit_lora_t(g, g2a):
            # transpose two tiles at a time: [128, 64] -> [64, 128], so the
            # per-tile rows sit at base partition 0/32 (PE quadrant rule)
            t0g, ngg = groups[g]
            lst = []
            for h0 in range(0, ngg, 2):
                w = min(2, ngg - h0) * 32
                pta = ps_t.tile([w, P], bf16, tag="pt", name=f"pt{g}_{h0}",
                                bufs=2)
                nc.tensor.matmul(pta[:], g2a[:, h0 * 32:h0 * 32 + w],
                                 ident_sb[:], is_transpose=True)
                gta = gtp.tile([w, P], bf16, tag="gt", name=f"gt{g}_{h0}")
                nc.vector.tensor_copy(gta[:], pta[:])
                for j in range(w // 32):
                    lst.append(gta[j * 32:j * 32 + ER, :])
            gts[g] = lst

        def emit_mm2_tile(g, i, actT):
            t0g, ngg = groups[g]
            G = ngg * P
            tt = t0g + i
            po = ps_o.tile([P, D], f32, tag="po", name=f"po{tt}")
            for h in range(NH):
                nc.tensor.matmul(
                    po[:],
                    actT[:, h * G + i * P:h * G + (i + 1) * P],
                    W2s[:, h * D:(h + 1) * D],
                    start=(h == 0), stop=False)
            nc.tensor.matmul(po[:], gts[g][i],
                             wur[(i % 2) * 32:(i % 2) * 32 + ER, :],
                             start=False, stop=True)
            o_sb = outp.tile([P, D], f32, tag="o_sb", name=f"o_sb{tt}")
            nc.vector.tensor_tensor(o_sb[:], po[:], b2rep[:], op=OP.add)
            nc.sync.dma_start(out[tt * P:(tt + 1) * P, :], o_sb[:])

        # ---- main loop, two-stage software pipeline on the PE:
        # MM1 of group g+1 is interleaved between MM2 tiles of group g, and
        # the LoRA chain for group g+1 also runs under MM2 of group g, so
        # neither a group boundary nor the final group ever stalls the PE.
        actTs = {}

        def make_actT(g):
            t0g, ngg = groups[g]
            a = actp.tile([P, NH * ngg * P], bf16, tag="actT", name=f"actT{g}")
            actTs[g] = a
            return a

        # PE warm-up: throwaway matmuls while the first DMAs land, so the
        # clock ramp (triggered by activity) completes before real work
        for k in range(5):
            pd = ps_o.tile([P, D], f32, tag="po", name=f"dum{k}")
            nc.tensor.matmul(pd[:], dum_in[:, 0:P], dum_in[:],
                             start=True, stop=True)

        # pipeline fill: MM1(0), with group 0's LoRA folded in at points
        # where its inputs (wd, routing weights) have arrived
        a0 = make_actT(0)
        for h in range(NH):
            emit_mm1_chunk(0, h, a0)
            if h == 6:
                g0_g2s = emit_lora_down(0)
            if h == 12:
                emit_lora_t(0, g0_g2s)

        for g in range(ngrp):
            t0g, ngg = groups[g]
            actT = actTs[g]
            if g + 1 < ngrp:
                nxt = make_actT(g + 1)
                nxt_g2s = emit_lora_down(g + 1)
                emit_mm2_tile(g, 0, actT)
                emit_lora_t(g + 1, nxt_g2s)
                blocks = ((0, 5), (5, 10), (10, 16))
                for bi, i in enumerate(range(1, ngg)):
                    for h in range(*blocks[bi]):
                        emit_mm1_chunk(g + 1, h, nxt)
                    emit_mm2_tile(g, i, actT)
            else:
                for i in range(ngg):
                    emit_mm2_tile(g, i, actT)
            if g + 2 < ngrp:
                dma_xt(g + 2)

    nc.compile()
    return nc


def _get_nc():
    if "nc" not in _CACHE:
        _CACHE["nc"] = _build()
    return _CACHE["nc"]


def _make_in_maps(inputs):
    import ml_dtypes
    bf16 = ml_dtypes.bfloat16

    x = np.asarray(inputs["x"], dtype=np.float32)
    T = x.size // D
    x_flat = x.reshape(T, D)
    W1 = np.asarray(inputs["W1"], dtype=np.float32)
    W2 = np.asarray(inputs["W2"], dtype=np.float32)
    b1 = np.asarray(inputs["b1"], dtype=np.float32)
    b2 = np.asarray(inputs["b2"], dtype=np.float32)
    wdn = np.asarray(inputs["w_down"], dtype=np.float32)
    wup = np.asarray(inputs["w_up"], dtype=np.float32)
    tkp = np.ascontiguousarray(inputs["topk_probs"], dtype=np.float32)
    tki_in = np.asarray(inputs["topk_indices"])

    # SBUF layouts, bf16 (shared by all cores)
    # W1 [D, DH] -> [p, q, hh, c, x]: hh-major so the first DMA piece covers
    # the first MM1 h-chunks
    w1p = np.ascontiguousarray(
        W1.reshape(NDC, P, 4, 4, P).transpose(1, 2, 3, 0, 4).reshape(P, 4 * DH)
    ).astype(bf16)
    # W2 [DH, D] -> [p, n, d]
    w2p = np.ascontiguousarray(
        W2.reshape(NH, P, D).transpose(1, 0, 2).reshape(P, NH * D)).astype(bf16)
    # w_down [E, D, R] -> [D, E*R] -> [p, c, er]
    wdf = wdn.transpose(1, 0, 2).reshape(D, ER)
    wdp = np.ascontiguousarray(
        wdf.reshape(NDC, P, ER).transpose(1, 0, 2).reshape(P, NDC * ER)
    ).astype(bf16)
    wup_b = np.zeros((P, D), dtype=bf16)
    for i in range(GT):
        wup_b[i * 32:i * 32 + ER, :] = wup.reshape(ER, D).astype(bf16)
    b1p = np.ascontiguousarray(b1.reshape(NH, P).T)
    b2p = np.ascontiguousarray(np.broadcast_to(b2[None, :], (P, D)),
                               dtype=np.float32)
    ident = np.eye(P, dtype=np.float32).astype(bf16)

    groups = []
    t0 = 0
    while t0 < NT:
        ng = min(GT, NT - t0)
        groups.append((t0, ng))
        t0 += ng

    in_maps = []
    for c in range(N_CORES):
        sl = slice(c * TC, (c + 1) * TC)
        # x.T in bf16, regrouped per supergroup to [p, (group: c, t)] so the
        # per-group DMA is one contiguous chunk per partition
        xc = x_flat[sl].T.astype(bf16)          # [D, TC] = [(c p), t]
        xr = xc.reshape(NDC, P, TC)
        parts = [
            np.ascontiguousarray(
                xr[:, :, t0g * P:(t0g + ngg) * P].transpose(1, 0, 2)
            ).reshape(P, -1)
            for t0g, ngg in groups
        ]
        xt_c = np.ascontiguousarray(np.concatenate(parts, axis=1))
        # routing tensors packed to [p, n, k] (token tile n, partition p)
        tkp_c = np.ascontiguousarray(
            tkp[sl].reshape(NT, P, 2).transpose(1, 0, 2)).reshape(P, NT * 2)
        tki4 = np.zeros((TC, 4), dtype=np.int32)
        tki4[:, 0] = tki_in[sl, 0]
        tki4[:, 2] = tki_in[sl, 1]
        tki_c = np.ascontiguousarray(
            tki4.reshape(NT, P, 4).transpose(1, 0, 2)).reshape(P, NT * 4)
        in_maps.append(dict(
            xt=xt_c, w1=w1p, w2=w2p, wd=wdp, wu=wup_b, b1=b1p, b2=b2p,
            tkp=tkp_c, tki=tki_c, ident=ident))
    return in_maps


def _ensure_ntff_hook():
    """Register the axon NTFF profile hook if the image's antenv lacks it."""
    import sys
    import types
    try:
        from antenv.axon_hooks import get_axon_ntff_profile_hook  # noqa: F401
        return True
    except ImportError:
        pass
    try:
        from trn_agent_boot.trn_boot import _ntff_profile_via_ctypes
        mod = types.ModuleType("antenv.axon_hooks")
        _hook = [None]
        mod.set_axon_ntff_profile_hook = lambda h: _hook.__setitem__(0, h)
        mod.get_axon_ntff_profile_hook = lambda: _hook[0]
        sys.modules["antenv.axon_hooks"] = mod
        import antenv
        antenv.axon_hooks = mod
        mod.set_axon_ntff_profile_hook(
            _ntff_profile_via_ctypes("/opt/axon/libaxon_pjrt.so"))
        return True
    except Exception:
        return False


def kernel(**inputs):
    from concourse.bass_utils import run_bass_kernel_spmd

    nc = _get_nc()
    in_maps = _make_in_maps(inputs)
    trace = bool(int(os.environ.get("KERNEL_TRACE", "0")))
    if trace and not _ensure_ntff_hook():
        trace = False
    res = run_bass_kernel_spmd(nc, in_maps, list(range(N_CORES)), trace=trace)
    if trace:
        _CACHE["last_result"] = res
    out = np.concatenate([res.results[i]["out"] for i in range(N_CORES)], axis=0)
    return out.reshape(np.asarray(inputs["x"]).shape).astype(np.float32)


# revision 27
# speedup vs baseline: 1.0060x; 1.0025x over previous
"""Trainium2 Bass kernel: ConvNeXt MLP + parallel MoE-LoRA (data-parallel over tokens).

Math per token t (D=512, Dh=2048, E=3 experts, r=8, top-k=2):
    base = gelu(x @ W1 + b1) @ W2 + b2
    g_e  = gelu(x @ w_down[e]) * wts[e, t]          (wts from top-k routing)
    out  = base + sum_e g_e @ w_up[e]

Strategy (per NeuronCore, 8 cores data-parallel on the token dim):
  - all operands are pre-packed on the host into their SBUF layouts in bf16:
    x arrives pre-transposed as xT [D, T_core] so the PE never runs x
    transposes and the DVE never runs casts; weights arrive in matmul-ready
    chunk layouts; b2 arrives pre-replicated [128, 512].
  - tokens tiled 128 at a time; supergroups of 4 tiles (512 tokens) so the
    MM1 moving free dim is 512.
  - MM1: hT[h,t] = W1_chunk.T @ xT (feature-major hidden, 4 PSUM banks so
    the Gelu drain never stalls the accumulation cadence), fused bias+gelu
    on ScalarE into actT (bf16).
  - MM2: out[t,d] accumulates 16 h-chunks (lhsT = actT slices) + the
    MoE-LoRA rank-24 matmul in one PSUM accumulation group; b2 added during
    the PSUM->SBUF drain from the preloaded replicated bias tile.
  - LoRA for supergroup g+1 is computed during MM2 of supergroup g (down
    proj -> gelu -> routing scale -> PE transpose), so the close matmul of
    MM2 never waits on the scalar/vector chain.
  - routing weights wts[e,t] = sum_k probs[t,k]*(idx[t,k]==e) computed on
    device in a small DVE prologue over all tokens at once.
  - W1 is DMA'd in h-quarter pieces so MM1 starts after ~1MB of weight
    traffic; the Gelu activation table is pre-warmed by a dummy activation.
"""

import os
import numpy as np

P = 128
D = 512
DH = 2048
E = 3
R = 8
ER = E * R  # 24
NH = DH // P  # 16
NDC = D // P  # 4 d-chunks
N_CORES = 8
T_FULL = 64 * 28 * 28  # 50176
TC = T_FULL // N_CORES  # 6272
NT = TC // P  # 49 token tiles
GT = 4  # tiles per supergroup

_CACHE = {}


def _build():
    import concourse.bacc as bacc
    import concourse.tile as tile
    import concourse.mybir as mybir
    from contextlib import ExitStack

    f32 = mybir.dt.float32
    bf16 = mybir.dt.bfloat16
    i32 = mybir.dt.int32
    AF = mybir.ActivationFunctionType
    OP = mybir.AluOpType

    nt = NT
    groups = []
    t0 = 0
    while t0 < nt:
        ng = min(GT, nt - t0)
        groups.append((t0, ng))
        t0 += ng
    ngrp = len(groups)

    nc = bacc.Bacc("TRN2", target_bir_lowering=False, debug=False,
                   num_devices=N_CORES)

    xt = nc.dram_tensor("xt", [P, NDC * TC], bf16, kind="ExternalInput").ap()
    w1 = nc.dram_tensor("w1", [P, 4 * DH], bf16, kind="ExternalInput").ap()
    w2 = nc.dram_tensor("w2", [P, NH * D], bf16, kind="ExternalInput").ap()
    wd = nc.dram_tensor("wd", [P, NDC * ER], bf16, kind="ExternalInput").ap()
    wu = nc.dram_tensor("wu", [P, D], bf16, kind="ExternalInput").ap()
    b1 = nc.dram_tensor("b1", [P, NH], f32, kind="ExternalInput").ap()
    b2 = nc.dram_tensor("b2", [P, D], f32, kind="ExternalInput").ap()
    tkp = nc.dram_tensor("tkp", [P, NT * 2], f32, kind="ExternalInput").ap()
    tki = nc.dram_tensor("tki", [P, NT * 4], i32, kind="ExternalInput").ap()
    ident_d = nc.dram_tensor("ident", [P, P], bf16, kind="ExternalInput").ap()
    out = nc.dram_tensor("out", [TC, D], f32, kind="ExternalOutput").ap()

    with tile.TileContext(nc) as tc, ExitStack() as ctx:
        cons = ctx.enter_context(tc.tile_pool(name="cons", bufs=1))
        xtp = ctx.enter_context(tc.tile_pool(name="xtp", bufs=4))
        actp = ctx.enter_context(tc.tile_pool(name="actp", bufs=2))
        outp = ctx.enter_context(tc.tile_pool(name="outp", bufs=4))
        lp = ctx.enter_context(tc.tile_pool(name="lp", bufs=4))
        gtp = ctx.enter_context(tc.tile_pool(name="gtp", bufs=8))
        ps_h = ctx.enter_context(tc.tile_pool(name="ps_h", bufs=4, space="PSUM"))
        ps_o = ctx.enter_context(tc.tile_pool(name="ps_o", bufs=2, space="PSUM"))
        ps_g = ctx.enter_context(tc.tile_pool(name="ps_g", bufs=1, space="PSUM"))
        ps_t = ctx.enter_context(tc.tile_pool(name="ps_t", bufs=1, space="PSUM"))

        # preload the Gelu activation table before the first real activation
        warm_in = cons.tile([1, 8], f32)
        nc.vector.memset(warm_in[:], 0.125)
        warm_out = cons.tile([1, 8], f32)
        nc.scalar.activation(warm_out[:], warm_in[:], AF.Gelu)
        # dummy matmul fodder: the PE clock ramps ~6us after sustained
        # activity starts, so burn the DMA wait on throwaway matmuls to
        # trigger the ramp before the real data lands
        dum_in = cons.tile([P, 512], bf16)
        nc.vector.memset(dum_in[:], 0.0)

        # ---- DMAs in issue order (earlier == higher effective priority).
        # All sources are host-packed so every DMA is one contiguous chunk
        # per partition (cheap descriptor generation on the sync engine).
        ident_sb = cons.tile([P, P], bf16)
        nc.sync.dma_start(ident_sb[:], ident_d)

        xT_tiles = {}

        def dma_xt(g):
            t0g, ngg = groups[g]
            G = ngg * P
            xTt = xtp.tile([P, NDC * G], bf16, tag="xT", name=f"xT{g}")
            nc.sync.dma_start(xTt[:], xt[:, NDC * t0g * P:NDC * (t0g + ngg) * P])
            xT_tiles[g] = xTt

        # first-needed data in small pieces so MM1 h-chunk 0 starts earliest:
        # W1 quarter 0 in four hh-pieces, xT group 0 in four c-pieces
        W1s = cons.tile([P, 4 * DH], bf16)
        xT0 = xtp.tile([P, NDC * GT * P], bf16, tag="xT", name="xT0")
        xT_tiles[0] = xT0
        b1s = cons.tile([P, NH], f32)
        wdr = cons.tile([P, NDC * ER], bf16)
        nc.sync.dma_start(W1s[:, 0:512], w1[:, 0:512])        # q0 hh0
        nc.sync.dma_start(xT0[:, 0:512], xt[:, 0:512])        # g0 c0
        nc.sync.dma_start(b1s[:], b1)
        nc.sync.dma_start(xT0[:, 512:1024], xt[:, 512:1024])  # g0 c1
        nc.sync.dma_start(W1s[:, 512:1024], w1[:, 512:1024])  # q0 hh1
        nc.sync.dma_start(xT0[:, 1024:1536], xt[:, 1024:1536])  # g0 c2
        nc.sync.dma_start(xT0[:, 1536:2048], xt[:, 1536:2048])  # g0 c3
        nc.sync.dma_start(W1s[:, 1024:2048], w1[:, 1024:2048])  # q0 hh2-3
        nc.sync.dma_start(wdr[:], wd)
        for q in (1, 2, 3):
            nc.sync.dma_start(W1s[:, q * DH:(q + 1) * DH],
                              w1[:, q * DH:(q + 1) * DH])
        tp_sb = cons.tile([P, nt * 2], f32)
        nc.sync.dma_start(tp_sb[:], tkp)
        ti_sb = cons.tile([P, nt * 4], i32)
        nc.sync.dma_start(ti_sb[:], tki)
        dma_xt(1)
        W2s = cons.tile([P, NH * D], bf16)
        for j in range(4):
            nc.sync.dma_start(W2s[:, j * 4 * D:(j + 1) * 4 * D],
                              w2[:, j * 4 * D:(j + 1) * 4 * D])
        # w_up replicated at partition offsets 0/32/64/96 so each tile's
        # close matmul has lhsT/rhs at the same (aligned) base partition
        wur = cons.tile([P, D], bf16)
        nc.sync.dma_start(wur[:], wu)
        b2rep = cons.tile([P, D], f32)
        nc.sync.dma_start(b2rep[:], b2)

        # ---- routing weights wts[p, e*nt + tile] (DVE) ----
        idxf = cons.tile([P, nt * 2], f32)
        iv = ti_sb.rearrange("p (n k two) -> p n k two", k=2, two=2)
        nc.vector.tensor_copy(
            idxf.rearrange("p (n k one) -> p n k one", k=2, one=1),
            iv[:, :, :, 0:1])
        wts = cons.tile([P, E * nt], f32)
        for e in range(E):
            eq = cons.tile([P, nt * 2], f32, tag="eq", name=f"eq{e}", bufs=2)
            nc.vector.tensor_scalar(eq[:], idxf[:], float(e), None,
                                    op0=OP.is_equal)
            nc.vector.tensor_tensor(eq[:], eq[:], tp_sb[:], op=OP.mult)
            nc.vector.tensor_reduce(wts[:, e * nt:(e + 1) * nt],
                                    eq.rearrange("p (n k) -> p n k", k=2),
                                    axis=mybir.AxisListType.X, op=OP.add)

        # ---- emit helpers ----
        gts = {}  # group -> list of [ER, P] bf16 lhsT tiles for the MM2 close

        def emit_mm1_chunk(g, h, actT):
            t0g, ngg = groups[g]
            G = ngg * P
            q, hh = h // 4, h % 4
            xTt = xT_tiles[g]
            ph = ps_h.tile([P, 512], f32, tag="ph", name=f"ph{g}_{h}")
            base = q * DH + hh * 512
            for c in range(NDC):
                nc.tensor.matmul(
                    ph[:, :G],
                    W1s[:, base + c * P:base + (c + 1) * P],
                    xTt[:, c * G:(c + 1) * G],
                    start=(c == 0), stop=(c == NDC - 1))
            nc.scalar.activation(actT[:, h * G:(h + 1) * G], ph[:, :G],
                                 AF.Gelu, bias=b1s[:, h:h + 1], scale=1.0)

        def emit_lora_down(g):
            t0g, ngg = groups[g]
            G = ngg * P
            xTt = xT_tiles[g]
            pga = ps_g.tile([P, GT * ER], f32, tag="pg", name=f"pg{g}")
            for i in range(ngg):
                for c in range(NDC):
                    nc.tensor.matmul(
                        pga[:, i * ER:(i + 1) * ER],
                        xTt[:, c * G + i * P:c * G + (i + 1) * P],
                        wdr[:, c * ER:(c + 1) * ER],
                        start=(c == 0), stop=(c == NDC - 1))
            g_sba = lp.tile([P, ngg * ER], f32, tag="g_sb", name=f"g_sb{g}")
            nc.scalar.activation(g_sba[:], pga[:, :ngg * ER], AF.Gelu)
            # g2 blocks padded to a 32-column stride so the transposed rows
            # land at base partitions 0/32/64/96 (PE tile-position rule)
            g2a = lp.tile([P, ngg * 32], bf16, tag="g2", name=f"g2{g}")
            for i in range(ngg):
                tt = t0g + i
                for e in range(E):
                    nc.vector.tensor_scalar(
                        g2a[:, i * 32 + e * R:i * 32 + (e + 1) * R],
                        g_sba[:, i * ER + e * R:i * ER + (e + 1) * R],
                        wts[:, e * nt + tt:e * nt + tt + 1], None,
                        op0=OP.mult)
            return g2a

        def emit_lora_t(g, g2a):
            # transpose two tiles at a time: [128, 64] -> [64, 128], so the
            # per-tile rows sit at base partition 0/32 (PE quadrant rule)
            t0g, ngg = groups[g]
            lst = []
            for h0 in range(0, ngg, 2):
                w = min(2, ngg - h0) * 32
                pta = ps_t.tile([w, P], bf16, tag="pt", name=f"pt{g}_{h0}",
                                bufs=1)
                nc.tensor.matmul(pta[:], g2a[:, h0 * 32:h0 * 32 + w],
                                 ident_sb[:], is_transpose=True)
                gta = gtp.tile([w, P], bf16, tag="gt", name=f"gt{g}_{h0}")
                nc.vector.tensor_copy(gta[:], pta[:])
                for j in range(w // 32):
                    lst.append(gta[j * 32:j * 32 + ER, :])
            gts[g] = lst

        def emit_mm2_tile(g, i, actT):
            t0g, ngg = groups[g]
            G = ngg * P
            tt = t0g + i
            po = ps_o.tile([P, D], f32, tag="po", name=f"po{tt}")
            for h in range(NH):
                nc.tensor.matmul(
                    po[:],
                    actT[:, h * G + i * P:h * G + (i + 1) * P],
                    W2s[:, h * D:(h + 1) * D],
                    start=(h == 0), stop=False)
            nc.tensor.matmul(po[:], gts[g][i],
                             wur[(i % 2) * 32:(i % 2) * 32 + ER, :],
                             start=False, stop=True)
            o_sb = outp.tile([P, D], f32, tag="o_sb", name=f"o_sb{tt}")
            nc.vector.tensor_tensor(o_sb[:], po[:], b2rep[:], op=OP.add)
            nc.sync.dma_start(out[tt * P:(tt + 1) * P, :], o_sb[:])

        # ---- main loop, two-stage software pipeline on the PE:
        # MM1 of group g+1 is interleaved between MM2 tiles of group g, and
        # the LoRA chain for group g+1 also runs under MM2 of group g, so
        # neither a group boundary nor the final group ever stalls the PE.
        actTs = {}

        def make_actT(g):
            t0g, ngg = groups[g]
            a = actp.tile([P, NH * ngg * P], bf16, tag="actT", name=f"actT{g}")
            actTs[g] = a
            return a

        # PE warm-up: throwaway matmuls while the first DMAs land, so the
        # clock ramp (triggered by activity) completes before real work
        for k in range(5):
            pd = ps_o.tile([P, D], f32, tag="po", name=f"dum{k}")
            nc.tensor.matmul(pd[:], dum_in[:, 0:P], dum_in[:],
                             start=True, stop=True)

        # pipeline fill: MM1(0), with group 0's LoRA folded in at points
        # where its inputs (wd, routing weights) have arrived
        a0 = make_actT(0)
        for h in range(NH):
            emit_mm1_chunk(0, h, a0)
            if h == 6:
                g0_g2s = emit_lora_down(0)
            if h == 12:
                emit_lora_t(0, g0_g2s)

        for g in range(ngrp):
            t0g, ngg = groups[g]
            actT = actTs[g]
            if g + 1 < ngrp:
                nxt = make_actT(g + 1)
                nxt_g2s = emit_lora_down(g + 1)
                emit_mm2_tile(g, 0, actT)
                emit_lora_t(g + 1, nxt_g2s)
                blocks = ((0, 5), (5, 10), (10, 16))
                for bi, i in enumerate(range(1, ngg)):
                    for h in range(*blocks[bi]):
                        emit_mm1_chunk(g + 1, h, nxt)
                    emit_mm2_tile(g, i, actT)
            else:
                for i in range(ngg):
                    emit_mm2_tile(g, i, actT)
            if g + 2 < ngrp:
                dma_xt(g + 2)

    nc.compile()
    return nc


def _get_nc():
    if "nc" not in _CACHE:
        _CACHE["nc"] = _build()
    return _CACHE["nc"]


def _make_in_maps(inputs):
    import ml_dtypes
    bf16 = ml_dtypes.bfloat16

    x = np.asarray(inputs["x"], dtype=np.float32)
    T = x.size // D
    x_flat = x.reshape(T, D)
    W1 = np.asarray(inputs["W1"], dtype=np.float32)
    W2 = np.asarray(inputs["W2"], dtype=np.float32)
    b1 = np.asarray(inputs["b1"], dtype=np.float32)
    b2 = np.asarray(inputs["b2"], dtype=np.float32)
    wdn = np.asarray(inputs["w_down"], dtype=np.float32)
    wup = np.asarray(inputs["w_up"], dtype=np.float32)
    tkp = np.ascontiguousarray(inputs["topk_probs"], dtype=np.float32)
    tki_in = np.asarray(inputs["topk_indices"])

    # SBUF layouts, bf16 (shared by all cores)
    # W1 [D, DH] -> [p, q, hh, c, x]: hh-major so the first DMA piece covers
    # the first MM1 h-chunks
    w1p = np.ascontiguousarray(
        W1.reshape(NDC, P, 4, 4, P).transpose(1, 2, 3, 0, 4).reshape(P, 4 * DH)
    ).astype(bf16)
    # W2 [DH, D] -> [p, n, d]
    w2p = np.ascontiguousarray(
        W2.reshape(NH, P, D).transpose(1, 0, 2).reshape(P, NH * D)).astype(bf16)
    # w_down [E, D, R] -> [D, E*R] -> [p, c, er]
    wdf = wdn.transpose(1, 0, 2).reshape(D, ER)
    wdp = np.ascontiguousarray(
        wdf.reshape(NDC, P, ER).transpose(1, 0, 2).reshape(P, NDC * ER)
    ).astype(bf16)
    wup_b = np.zeros((P, D), dtype=bf16)
    for i in range(GT):
        wup_b[i * 32:i * 32 + ER, :] = wup.reshape(ER, D).astype(bf16)
    b1p = np.ascontiguousarray(b1.reshape(NH, P).T)
    b2p = np.ascontiguousarray(np.broadcast_to(b2[None, :], (P, D)),
                               dtype=np.float32)
    ident = np.eye(P, dtype=np.float32).astype(bf16)

    groups = []
    t0 = 0
    while t0 < NT:
        ng = min(GT, NT - t0)
        groups.append((t0, ng))
        t0 += ng

    in_maps = []
    for c in range(N_CORES):
        sl = slice(c * TC, (c + 1) * TC)
        # x.T in bf16, regrouped per supergroup to [p, (group: c, t)] so the
        # per-group DMA is one contiguous chunk per partition
        xc = x_flat[sl].T.astype(bf16)          # [D, TC] = [(c p), t]
        xr = xc.reshape(NDC, P, TC)
        parts = [
            np.ascontiguousarray(
                xr[:, :, t0g * P:(t0g + ngg) * P].transpose(1, 0, 2)
            ).reshape(P, -1)
            for t0g, ngg in groups
        ]
        xt_c = np.ascontiguousarray(np.concatenate(parts, axis=1))
        # routing tensors packed to [p, n, k] (token tile n, partition p)
        tkp_c = np.ascontiguousarray(
            tkp[sl].reshape(NT, P, 2).transpose(1, 0, 2)).reshape(P, NT * 2)
        tki4 = np.zeros((TC, 4), dtype=np.int32)
        tki4[:, 0] = tki_in[sl, 0]
        tki4[:, 2] = tki_in[sl, 1]
        tki_c = np.ascontiguousarray(
            tki4.reshape(NT, P, 4).transpose(1, 0, 2)).reshape(P, NT * 4)
        in_maps.append(dict(
            xt=xt_c, w1=w1p, w2=w2p, wd=wdp, wu=wup_b, b1=b1p, b2=b2p,
            tkp=tkp_c, tki=tki_c, ident=ident))
    return in_maps


def _ensure_ntff_hook():
    """Register the axon NTFF profile hook if the image's antenv lacks it."""
    import sys
    import types
    try:
        from antenv.axon_hooks import get_axon_ntff_profile_hook  # noqa: F401
        return True
    except ImportError:
        pass
    try:
        from trn_agent_boot.trn_boot import _ntff_profile_via_ctypes
        mod = types.ModuleType("antenv.axon_hooks")
        _hook = [None]
        mod.set_axon_ntff_profile_hook = lambda h: _hook.__setitem__(0, h)
        mod.get_axon_ntff_profile_hook = lambda: _hook[0]
        sys.modules["antenv.axon_hooks"] = mod
        import antenv
        antenv.axon_hooks = mod
        mod.set_axon_ntff_profile_hook(
            _ntff_profile_via_ctypes("/opt/axon/libaxon_pjrt.so"))
        return True
    except Exception:
        return False


def kernel(**inputs):
    from concourse.bass_utils import run_bass_kernel_spmd

    nc = _get_nc()
    in_maps = _make_in_maps(inputs)
    trace = bool(int(os.environ.get("KERNEL_TRACE", "0")))
    if trace and not _ensure_ntff_hook():
        trace = False
    res = run_bass_kernel_spmd(nc, in_maps, list(range(N_CORES)), trace=trace)
    if trace:
        _CACHE["last_result"] = res
    out = np.concatenate([res.results[i]["out"] for i in range(N_CORES)], axis=0)
    return out.reshape(np.asarray(inputs["x"]).shape).astype(np.float32)
